# revision 1
# baseline (speedup 1.0000x reference)
"""Trainium2 Bass kernel for a dense transformer block.

Reference math (B=32, S=577, D=768, H=12, DH=64, F=3072, fp32):
  h  = LN1(x);  q,k,v = per-head projections of h
  scores = q @ k^T / sqrt(DH)
  probs  = softmax(scores, axis=QUERY)       # quirk: softmax over the query axis
  attn   = probs @ v;  x2 = x + concat(attn) @ Wo + bo
  out    = x2 + (gelu(LN2(x2) @ W1 + b1) @ W2 + b2)

Strategy: pure data-parallel over batch, 4 batch items per core on 8 cores, no
collectives.  All on-chip activations live in a transposed layout [feature on
partitions, token on free dim].  The attention path (QKV projections, probs@V,
output projection) runs in fp8e4 with DoubleRow perf mode: weights are scaled
x64 and cast to fp8 on the host, activations are quantized on the fly, and
each DoubleRow matmul consumes a 256-deep contraction (two 128-chunks in the
free-dim pair slots), doubling PE throughput.  Scores stay bf16 (64-deep
contraction can't pair).  The MLP is bf16 by default (MLP_MODE="fc1" switches
FC1 to fp8 DoubleRow too).  Residual stream stays fp32.
"""

import numpy as np
import ml_dtypes

B, S, D, H, DH, F = 32, 577, 768, 12, 64, 3072
NCORES = 8
BPC = B // NCORES          # batches per core
EPS = 1e-5
NCD = D // 128             # 6  d-chunks
NCF = F // 128             # 24 f-chunks
NHP = H // 2               # 6  head pairs
SSPL = [(0, 512), (512, S - 512)]              # free-dim splits of S for matmul/psum
DSPL = [(0, 512), (512, D - 512)]              # free-dim splits of D
TCH = [(i * 128, min(128, S - i * 128)) for i in range((S + 127) // 128)]  # 5 t-chunks
SP = 640                   # fp8 tile row pitch: DoubleRow ldweights needs stride % 64 == 0

W_SC = 64.0      # fp8 weight scale
QK_SC = 64.0     # q/k live at 64x in bf16; folded into the exp input scale
V_SC = 1024.0    # v/rowsum product scale for fp8 storage
C_SC = 32.0      # concat (attn output) fp8 scale

MLP_MODE = "fc1"   # "bf16" | "fc1"

_NC_CACHE = {}


def _build_nc(gelu_kind: str = "gelu", bpc: int = BPC, mlp_mode: str = MLP_MODE, no_bias: bool = False):
    from contextlib import ExitStack
    import concourse.bass as bass
    import concourse.tile as tile
    from concourse import bacc, mybir

    f32, bf16 = mybir.dt.float32, mybir.dt.bfloat16
    f8 = mybir.dt.float8e4
    AF = mybir.ActivationFunctionType
    ALU = mybir.AluOpType
    DR = mybir.MatmulPerfMode.DoubleRow
    GELU = {"gelu": AF.Gelu, "tanh": AF.Tanh}[gelu_kind]
    fc18 = (mlp_mode == "fc1")
    w1dt = f8 if fc18 else bf16
    z2dt = f8 if fc18 else bf16

    nc = bacc.Bacc("TRN2", target_bir_lowering=False, dynamic_dma_scratch_size=2048)
    xT_d = nc.declare_dram_parameter("xT", [bpc, D, S], f32, isOutput=False)
    wq_d = nc.declare_dram_parameter("wq", [D, D], f8, isOutput=False)
    wk_d = nc.declare_dram_parameter("wk", [D, D], f8, isOutput=False)
    wv_d = nc.declare_dram_parameter("wv", [D, D], f8, isOutput=False)
    wo_d = nc.declare_dram_parameter("wo", [D, D], f8, isOutput=False)
    w1_d = nc.declare_dram_parameter("w1", [D, F], w1dt, isOutput=False)
    w2_d = nc.declare_dram_parameter("w2", [F, D], bf16, isOutput=False)
    bq_d = nc.declare_dram_parameter("bq", [NCD, 128], f32, isOutput=False)
    bk_d = nc.declare_dram_parameter("bk", [NCD, 128], f32, isOutput=False)
    bv_d = nc.declare_dram_parameter("bv", [1, D], bf16, isOutput=False)
    bo_d = nc.declare_dram_parameter("bo", [1, D], bf16, isOutput=False)
    b1_d = nc.declare_dram_parameter("b1", [NCF, 128], f32, isOutput=False)
    b2_d = nc.declare_dram_parameter("b2", [NCD, 128], f32, isOutput=False)
    outT_d = nc.declare_dram_parameter("outT", [bpc, D, S], f32, isOutput=True)

    with tile.TileContext(nc) as tc:
        with ExitStack() as ctx:
            wp = ctx.enter_context(tc.tile_pool(name="wp", bufs=1))
            rp = ctx.enter_context(tc.tile_pool(name="rp", bufs=2))      # residual f32
            zp = ctx.enter_context(tc.tile_pool(name="zp", bufs=1))      # normalized
            qkp = ctx.enter_context(tc.tile_pool(name="qkp", bufs=1))    # qt/kt/v/concat
            ep = ctx.enter_context(tc.tile_pool(name="ep", bufs=2))      # exp tiles
            gp = ctx.enter_context(tc.tile_pool(name="gp", bufs=1))      # gelu acts
            sp_ = ctx.enter_context(tc.tile_pool(name="sp", bufs=1))     # small stat rows
            tp = ctx.enter_context(tc.tile_pool(name="tp", bufs=1))      # [128,S] temps
            mmp = ctx.enter_context(tc.tile_pool(name="mmp", bufs=4, space="PSUM"))

            # ---- weights / constants (resident); DMAs deferred until after
            # the first x-shard load so compute starts immediately ----
            wq_s = wp.tile([128, NCD, D], f8, name="wq_s")
            wk_s = wp.tile([128, NCD, D], f8, name="wk_s")
            wv_s = wp.tile([128, NCD, D], f8, name="wv_s")
            wo_s = wp.tile([128, NCD, D], f8, name="wo_s")
            w1_s = wp.tile([128, NCD, F], w1dt, name="w1_s")
            w2_s = wp.tile([128, NCF, D], bf16, name="w2_s")

            def emit_load_weights():
                nc.sync.dma_start(out=wq_s[:, :, :], in_=wq_d.ap().rearrange("(c p) n -> p c n", p=128))
                nc.sync.dma_start(out=wk_s[:, :, :], in_=wk_d.ap().rearrange("(c p) n -> p c n", p=128))
                nc.sync.dma_start(out=wv_s[:, :, :], in_=wv_d.ap().rearrange("(c p) n -> p c n", p=128))
                nc.sync.dma_start(out=wo_s[:, :, :], in_=wo_d.ap().rearrange("(c p) n -> p c n", p=128))
                nc.sync.dma_start(out=w1_s[:, :, :], in_=w1_d.ap().rearrange("(c p) n -> p c n", p=128))
                nc.sync.dma_start(out=w2_s[:, :, :], in_=w2_d.ap().rearrange("(c p) n -> p c n", p=128))
            bqs = wp.tile([128, NCD], f32, name="bqs")
            nc.sync.dma_start(out=bqs[:, :], in_=bq_d.ap().rearrange("c p -> p c"))
            bks = wp.tile([128, NCD], f32, name="bks")
            nc.sync.dma_start(out=bks[:, :], in_=bk_d.ap().rearrange("c p -> p c"))
            bvs = wp.tile([1, D], bf16, name="bvs")
            nc.sync.dma_start(out=bvs[:, :], in_=bv_d[:, :])
            bos = wp.tile([1, D], bf16, name="bos")
            nc.sync.dma_start(out=bos[:, :], in_=bo_d[:, :])
            b1s = wp.tile([128, NCF], f32, name="b1s")
            nc.sync.dma_start(out=b1s[:, :], in_=b1_d.ap().rearrange("c p -> p c"))
            b2s = wp.tile([128, NCD], f32, name="b2s")
            nc.sync.dma_start(out=b2s[:, :], in_=b2_d.ap().rearrange("c p -> p c"))
            ones128 = wp.tile([128, 1], bf16, name="ones128")
            nc.vector.memset(ones128[:, :], 1.0)
            ones128f = wp.tile([128, 1], f32, name="ones128f")
            nc.vector.memset(ones128f[:, :], 1.0)
            ones1 = wp.tile([1, 128], bf16, name="ones1")
            nc.vector.memset(ones1[:, :], 1.0)
            onesS = wp.tile([1, 512], bf16, name="onesS")
            nc.vector.memset(onesS[:, :], 1.0)
            eps_s = wp.tile([1, 1], f32, name="eps_s")
            nc.vector.memset(eps_s[:, :], EPS)

            # ---------------- helpers ----------------
            def emit_squares(src, c):
                """Square of one chunk of src -> bf16 tile for sumsq (on the
                otherwise idle GpSimd engine to keep ACT free for exp/gelu)."""
                sq = tp.tile([128, S], bf16, name="sq", tag="castsq", bufs=2)
                nc.gpsimd.tensor_mul(sq[:, :], src[:, c, :], src[:, c, :])
                return sq

            def emit_cast(src, c):
                """DVE bf16 copy of one chunk so the sum matmul runs at
                1 cyc/col instead of fp32's 4 (mean error from bf16 ~1e-4)."""
                xb = tp.tile([128, S], bf16, name="xb", tag="xcast", bufs=2)
                nc.vector.tensor_scalar_mul(xb[:, :], src[:, c, :], 1.0)
                return xb

            def emit_stats(src, sqs=None):
                """Column sums & sums of squares of src [128, NCD, S] f32 over
                the partition (feature) axis -> psum rows [0]=sum, [32]=sumsq."""
                spt = mmp.tile([128, S], f32, name="spt", tag="mm", padded_shape=[128, 1024])
                if sqs is None:
                    sqs = [emit_squares(src, c) for c in range(NCD)]
                xbs = [emit_cast(src, c) for c in range(NCD)]
                for c in range(NCD):
                    for (s0, sn) in SSPL:
                        nc.tensor.matmul(spt[0:1, s0:s0 + sn], ones128[:, :],
                                         xbs[c][:, s0:s0 + sn],
                                         start=(c == 0), stop=(c == NCD - 1))
                        nc.tensor.matmul(spt[32:33, s0:s0 + sn], ones128[:, :],
                                         sqs[c][:, s0:s0 + sn],
                                         start=(c == 0), stop=(c == NCD - 1))
                return spt

            def emit_chain(spt):
                """LN scalar chain on [1,S] rows."""
                mu_s = sp_.tile([1, S], f32, name="mu_s", tag="mu_s")
                nc.vector.tensor_scalar_mul(mu_s[:, :], spt[0:1, :], 1.0 / D)
                v_s = sp_.tile([1, S], f32, name="v_s", tag="v_s")
                nc.scalar.activation(v_s[:, :], spt[32:33, :], AF.Copy, scale=1.0 / D)
                nc.vector.tensor_mul(spt[0:1, :], mu_s[:, :], mu_s[:, :])
                nc.vector.tensor_sub(v_s[:, :], v_s[:, :], spt[0:1, :])
                w_s = tp.tile([1, S], f32, name="w_s", tag="castsq", bufs=2)
                nc.scalar.activation(w_s[:, :], v_s[:, :], AF.Sqrt, bias=eps_s[0:1, 0:1])
                nc.vector.reciprocal_approx_fast(v_s[:, :], w_s[:, :])
                rstd_bf = sp_.tile([1, S], bf16, name="rstd_bf", tag="rstdbf")
                nc.scalar.activation(rstd_bf[:, :], v_s[:, :], AF.Copy)
                nmr_bf = sp_.tile([1, S], bf16, name="nmr_bf", tag="nmrbf")
                nc.vector.scalar_tensor_tensor(nmr_bf[:, :], mu_s[:, :], -1.0, v_s[:, :],
                                               op0=ALU.mult, op1=ALU.mult)
                return rstd_bf, nmr_bf

            def emit_bcast(row_bf):
                """Broadcast a [1,S] bf16 row across 128 partitions via rank-1 matmul."""
                bc = mmp.tile([128, D], f32, name="bc", tag="mm")
                for (s0, sn) in SSPL:
                    nc.tensor.matmul(bc[:, s0:s0 + sn], ones1[0:1, :], row_bf[0:1, s0:s0 + sn],
                                     start=True, stop=True)
                return bc

            def emit_znorm(src, rstd_bc, nmr_bc, z, zdt_is_f8):
                """z[:,c,:] = (src[:,c,:] * rstd) + negmurstd."""
                for c in range(NCD):
                    if zdt_is_f8:
                        t = tp.tile([128, S], bf16, name="zt", tag="zt", bufs=2)
                        nc.vector.tensor_mul(t[:, :], src[:, c, :], rstd_bc[:, 0:S])
                        nc.vector.tensor_add(z[:, c, 0:S], t[:, :], nmr_bc[:, 0:S])
                    else:
                        nc.vector.tensor_mul(z[:, c, 0:S], src[:, c, :], rstd_bc[:, 0:S])
                        nc.vector.tensor_add(z[:, c, 0:S], z[:, c, 0:S], nmr_bc[:, 0:S])

            # ---------------- phase emitters ----------------
            xt_tiles = [None] * bpc
            z1_tiles = [None] * bpc
            qt_tiles = [None] * bpc
            kt_tiles = [None] * bpc
            v_tiles = [None] * bpc
            v8_tiles = [None] * bpc
            z2_tiles = [None] * bpc
            x2_tiles = [None] * bpc
            ch1 = [None] * bpc
            ch2 = [None] * bpc

            def emit_load_x(b):
                xt = rp.tile([128, NCD, S], f32, name="xt", tag="res")
                for c in range(NCD):
                    nc.sync.dma_start(out=xt[:, c, :],
                                      in_=xT_d[b, c * 128:(c + 1) * 128, :])
                xt_tiles[b] = xt

            def emit_stats1(b):
                ch1[b] = emit_chain(emit_stats(xt_tiles[b]))

            def emit_zfinish1(b):
                rstd_bf, nmr_bf = ch1[b]
                rbc = emit_bcast(rstd_bf)
                nbc = emit_bcast(nmr_bf)
                # one pad column so the last v-projection t-chunk can run as an
                # even-M (66) DoubleRow ldweights; its extra output row is unused
                z1 = zp.tile([128, NCD, SP], f8, name="z1", tag="z1")
                nc.vector.memset(z1[:, :, S:S + 1], 0.0)
                emit_znorm(xt_tiles[b], rbc, nbc, z1, True)
                z1_tiles[b] = z1

            def emit_qkv(b):
                """QKV projections; emitted early (inside the previous batch's
                FC1 stretch) so scores/exp can start the moment FC1 drains."""
                z1 = z1_tiles[b]
                qt = qkp.tile([128, NHP, S], bf16, name="qt", tag="qt")
                kt = qkp.tile([128, NHP, S], bf16, name="kt", tag="kt")
                for hp in range(NHP):
                    hc = slice(hp * 128, (hp + 1) * 128)
                    qps = mmp.tile([128, S], f32, name="qps", tag="mm")
                    for (s0, sn) in SSPL:
                        for j in range(NCD // 2):
                            nc.tensor.matmul(qps[:, s0:s0 + sn],
                                             wq_s[:, 2 * j:2 * j + 2, hc],
                                             z1[:, 2 * j:2 * j + 2, s0:s0 + sn],
                                             start=(j == 0), stop=(j == NCD // 2 - 1),
                                             perf_mode=DR)
                    nc.vector.tensor_scalar_add(qt[:, hp, :], qps[:, 0:S], bqs[:, hp:hp + 1])
                    kps = mmp.tile([128, S], f32, name="kps", tag="mm")
                    for (s0, sn) in SSPL:
                        for j in range(NCD // 2):
                            nc.tensor.matmul(kps[:, s0:s0 + sn],
                                             wk_s[:, 2 * j:2 * j + 2, hc],
                                             z1[:, 2 * j:2 * j + 2, s0:s0 + sn],
                                             start=(j == 0), stop=(j == NCD // 2 - 1),
                                             perf_mode=DR)
                    nc.vector.tensor_scalar_add(kt[:, hp, :], kps[:, 0:S], bks[:, hp:hp + 1])
                # V in natural layout [t, v]; psum holds 64*v, kept in bf16
                v = qkp.tile([128, len(TCH), D], bf16, name="v", tag="v")
                v8 = qkp.tile([128, len(TCH), D], f8, name="v8", tag="v8")
                for it, (t0, tw) in enumerate(TCH):
                    twe = tw + (tw % 2)          # even-M ldweights (pad col is zero)
                    vps = mmp.tile([128, D], f32, name="vps", tag="mm")
                    for (s0, sn) in DSPL:
                        for j in range(NCD // 2):
                            nc.tensor.matmul(vps[0:twe, s0:s0 + sn],
                                             z1[:, 2 * j:2 * j + 2, t0:t0 + twe],
                                             wv_s[:, 2 * j:2 * j + 2, s0:s0 + sn],
                                             start=(j == 0),
                                             stop=(no_bias and j == NCD // 2 - 1),
                                             perf_mode=DR)
                        if not no_bias:
                            nc.tensor.matmul(vps[0:twe, s0:s0 + sn], ones1[0:1, 0:twe],
                                             bvs[0:1, s0:s0 + sn], start=False, stop=True)
                    nc.vector.tensor_scalar_mul(v[0:tw, it, :], vps[0:tw, 0:D], 1.0)
                qt_tiles[b], kt_tiles[b] = qt, kt
                v_tiles[b], v8_tiles[b] = v, v8

            def emit_attn_heads(b, interleave=()):
                """interleave: callables emitted between head-pairs to feed the
                PE while ACT grinds the softmax exps."""
                interleave = list(interleave)
                qt, kt = qt_tiles[b], kt_tiles[b]
                v, v8 = v_tiles[b], v8_tiles[b]
                # --- per-head-pair attention ---
                concat = qkp.tile([128, NCD, SP], f8, name="concat", tag="concat")
                for hp in range(NHP):
                    etiles = [None, None]
                    rstiles = [None, None]
                    for h2 in range(2):
                        hb = h2 * 64
                        e = ep.tile([128, len(TCH), SP], f8, name="e", tag="e", bufs=4)
                        rs = sp_.tile([128, len(TCH)], f32, name="rs", tag="rs", bufs=4)
                        for it, (t0, tw) in enumerate(TCH):
                            stps = mmp.tile([128, S], f32, name="stps", tag="mm")
                            for (s0, sn) in SSPL:
                                nc.tensor.matmul(stps[0:tw, s0:s0 + sn],
                                                 kt[hb:hb + 64, hp, t0:t0 + tw],
                                                 qt[hb:hb + 64, hp, s0:s0 + sn],
                                                 start=True, stop=True)
                            nc.scalar.activation(e[0:tw, it, 0:S], stps[0:tw, 0:S], AF.Exp,
                                                 bias=0.0,
                                                 scale=float(1.0 / (np.sqrt(DH) * QK_SC * QK_SC)),
                                                 accum_out=rs[0:tw, it:it + 1])
                        etiles[h2] = e
                        rstiles[h2] = rs
                    ap_ps = mmp.tile([128, S], f32, name="ap_ps", tag="mm",
                                     padded_shape=[128, 1024])
                    for h2 in range(2):
                        hb = h2 * 64
                        e, rs = etiles[h2], rstiles[h2]
                        rec = sp_.tile([128, len(TCH)], f32, name="rec", tag="rec", bufs=2)
                        nfull = len(TCH) - 1
                        # v8 = (64*v) * rec * (V_SC/64)  ->  fp8 of V_SC*v/rs
                        nc.vector.reciprocal(rec[:, 0:nfull], rs[:, 0:nfull])
                        nc.vector.tensor_scalar_mul(rec[:, 0:nfull], rec[:, 0:nfull],
                                                    float(V_SC / W_SC))
                        lt0, ltw = TCH[-1]
                        nc.vector.reciprocal(rec[0:ltw, nfull:nfull + 1],
                                             rs[0:ltw, nfull:nfull + 1])
                        nc.vector.tensor_scalar_mul(rec[0:ltw, nfull:nfull + 1],
                                                    rec[0:ltw, nfull:nfull + 1],
                                                    float(V_SC / W_SC))
                        seg = slice(hp * 128 + hb, hp * 128 + hb + 64)
                        for it, (t0, tw) in enumerate(TCH):
                            nc.vector.tensor_scalar_mul(v8[0:tw, it, seg],
                                                        v[0:tw, it, seg],
                                                        rec[0:tw, it:it + 1])
                        # DoubleRow dst must start at partition 0, so only the
                        # even head of each pair can pair t-chunks
                        for (s0, sn) in SSPL:
                            if hb == 0:
                                for jt in range(2):
                                    nc.tensor.matmul(ap_ps[hb:hb + 64, s0:s0 + sn],
                                                     v8[:, 2 * jt:2 * jt + 2, seg],
                                                     e[:, 2 * jt:2 * jt + 2, s0:s0 + sn],
                                                     start=(jt == 0), stop=False, perf_mode=DR)
                            else:
                                for it in range(len(TCH) - 1):
                                    nc.tensor.matmul(ap_ps[hb:hb + 64, s0:s0 + sn],
                                                     v8[:, it, seg],
                                                     e[:, it, s0:s0 + sn],
                                                     start=(it == 0), stop=False)
                            lt0, ltw = TCH[-1]
                            nc.tensor.matmul(ap_ps[hb:hb + 64, s0:s0 + sn],
                                             v8[0:ltw, len(TCH) - 1, seg],
                                             e[0:ltw, len(TCH) - 1, s0:s0 + sn],
                                             start=False, stop=True)
                    nc.vector.tensor_scalar_mul(concat[:, hp, 0:S], ap_ps[:, 0:S],
                                                float(C_SC / V_SC))
                    if interleave:
                        interleave.pop(0)()

                # --- output projection + residual; LN2 stats matmuls fused ---
                x2 = rp.tile([128, NCD, S], f32, name="x2", tag="res")
                spt = mmp.tile([128, S], f32, name="spt2", tag="mm", padded_shape=[128, 1024])
                for ec in range(NCD):
                    xres = tp.tile([128, S], f32, name="xres", tag="castsq", bufs=2)
                    nc.sync.dma_start(out=xres[:, :],
                                      in_=xT_d[b, ec * 128:(ec + 1) * 128, :])
                    wops = mmp.tile([128, S], f32, name="wops", tag="mm")
                    for (s0, sn) in SSPL:
                        for j in range(NCD // 2):
                            nc.tensor.matmul(wops[:, s0:s0 + sn],
                                             wo_s[:, 2 * j:2 * j + 2, ec * 128:(ec + 1) * 128],
                                             concat[:, 2 * j:2 * j + 2, s0:s0 + sn],
                                             start=(j == 0),
                                             stop=(no_bias and j == NCD // 2 - 1),
                                             perf_mode=DR)
                        if not no_bias:
                            # + bo * (C_SC*W_SC) via rank-1
                            nc.tensor.matmul(wops[:, s0:s0 + sn],
                                             bos[0:1, ec * 128:(ec + 1) * 128],
                                             onesS[0:1, 0:sn], start=False, stop=True)
                    nc.vector.scalar_tensor_tensor(x2[:, ec, :], wops[:, 0:S],
                                                   float(1.0 / (C_SC * W_SC)), xres[:, :],
                                                   op0=ALU.mult, op1=ALU.add)
                    sq = emit_squares(x2, ec)
                    xb = emit_cast(x2, ec)
                    for (s0, sn) in SSPL:
                        nc.tensor.matmul(spt[0:1, s0:s0 + sn], ones128[:, :],
                                         xb[:, s0:s0 + sn],
                                         start=(ec == 0), stop=(ec == NCD - 1))
                        nc.tensor.matmul(spt[32:33, s0:s0 + sn], ones128[:, :],
                                         sq[:, s0:s0 + sn],
                                         start=(ec == 0), stop=(ec == NCD - 1))
                x2_tiles[b] = x2
                return spt

            def emit_stats2(b, spt):
                ch2[b] = emit_chain(spt)

            def emit_zfinish2(b):
                rstd_bf, nmr_bf = ch2[b]
                rbc = emit_bcast(rstd_bf)
                nbc = emit_bcast(nmr_bf)
                z2 = zp.tile([128, NCD, SP if fc18 else S], z2dt, name="z2", tag="z2")
                emit_znorm(x2_tiles[b], rbc, nbc, z2, fc18)
                z2_tiles[b] = z2

            def emit_fc1(b, g, lo, hi):
                z2 = z2_tiles[b]
                gsc = (1.0 / W_SC) if fc18 else 1.0
                for fc in range(lo, hi):
                    fps = mmp.tile([128, S], f32, name="fps", tag="mm")
                    if fc18:
                        for (s0, sn) in SSPL:
                            for j in range(NCD // 2):
                                nc.tensor.matmul(fps[:, s0:s0 + sn],
                                                 w1_s[:, 2 * j:2 * j + 2, fc * 128:(fc + 1) * 128],
                                                 z2[:, 2 * j:2 * j + 2, s0:s0 + sn],
                                                 start=(j == 0), stop=(j == NCD // 2 - 1),
                                                 perf_mode=DR)
                    else:
                        for (s0, sn) in SSPL:
                            for c in range(NCD):
                                nc.tensor.matmul(fps[:, s0:s0 + sn],
                                                 w1_s[:, c, fc * 128:(fc + 1) * 128],
                                                 z2[:, c, s0:s0 + sn],
                                                 start=(c == 0), stop=(c == NCD - 1))
                    nc.scalar.activation(g[:, fc, :], fps[:, 0:S], GELU,
                                         bias=b1s[:, fc:fc + 1], scale=gsc)

            def emit_fc2_chunk(b, g, ec):
                x2 = x2_tiles[b]
                p2 = mmp.tile([128, S], f32, name="p2", tag="mm")
                for (s0, sn) in SSPL:
                    for fc in range(NCF):
                        nc.tensor.matmul(p2[:, s0:s0 + sn],
                                         w2_s[:, fc, ec * 128:(ec + 1) * 128],
                                         g[:, fc, s0:s0 + sn],
                                         start=(fc == 0), stop=(fc == NCF - 1))
                nc.vector.scalar_tensor_tensor(x2[:, ec, :], p2[:, 0:S],
                                               b2s[:, ec:ec + 1], x2[:, ec, :],
                                               op0=ALU.add, op1=ALU.add)
                nc.sync.dma_start(out=outT_d[b, ec * 128:(ec + 1) * 128, :],
                                  in_=x2[:, ec, :])

            # ---------------- emission schedule ----------------
            emit_load_x(0)
            emit_load_weights()
            emit_stats1(0)
            emit_zfinish1(0)
            emit_qkv(0)
            spt2 = emit_attn_heads(0)
            emit_stats2(0, spt2)
            for b in range(bpc):
                if b + 1 < bpc:
                    emit_load_x(b + 1)
                emit_zfinish2(b)
                g = gp.tile([128, NCF, S], bf16, name="g", tag="g")
                emit_fc1(b, g, 0, 8)
                if b + 1 < bpc:
                    emit_stats1(b + 1)
                emit_fc1(b, g, 8, 14)
                if b + 1 < bpc:
                    emit_zfinish1(b + 1)
                emit_fc1(b, g, 14, 18)
                if b + 1 < bpc:
                    emit_qkv(b + 1)
                emit_fc1(b, g, 18, NCF)
                if b + 1 < bpc:
                    il = [(lambda ec=ec: emit_fc2_chunk(b, g, ec)) for ec in range(4)]
                    spt2 = emit_attn_heads(b + 1, interleave=il)
                    emit_stats2(b + 1, spt2)
                    emit_fc2_chunk(b, g, 4)
                    emit_fc2_chunk(b, g, 5)
                else:
                    for ec in range(NCD):
                        emit_fc2_chunk(b, g, ec)
    nc.finalize()
    return nc


def _get_nc(gelu_kind: str = "gelu", bpc: int = BPC, mlp_mode: str = MLP_MODE,
            no_bias: bool = False):
    key = (gelu_kind, bpc, mlp_mode, no_bias)
    if key not in _NC_CACHE:
        _NC_CACHE[key] = _build_nc(gelu_kind, bpc, mlp_mode, no_bias)
    return _NC_CACHE[key]


def _prep_weights(inputs, mlp_mode: str = MLP_MODE):
    bf16 = ml_dtypes.bfloat16
    f8 = ml_dtypes.float8_e4m3
    f32 = np.float32
    fc18 = (mlp_mode == "fc1")
    Wq, Wk, Wv = inputs["Wq"], inputs["Wk"], inputs["Wv"]
    g1, b1_ln = np.asarray(inputs["ln1_g"], f32), np.asarray(inputs["ln1_b"], f32)
    g2, b2_ln = np.asarray(inputs["ln2_g"], f32), np.asarray(inputs["ln2_b"], f32)

    def flat(Wx):  # [H, D, DH] -> [D, H*DH]
        return np.ascontiguousarray(np.transpose(np.asarray(Wx, f32), (1, 0, 2)).reshape(D, D))

    wq_f, wk_f, wv_f = flat(Wq), flat(Wk), flat(Wv)
    W1 = np.asarray(inputs["W1"], f32)
    w1_sc = (g2[:, None] * W1) * (W_SC if fc18 else 1.0)
    out = {
        "wq": np.ascontiguousarray((g1[:, None] * wq_f * W_SC).astype(f8)),
        "wk": np.ascontiguousarray((g1[:, None] * wk_f * W_SC).astype(f8)),
        "wv": np.ascontiguousarray((g1[:, None] * wv_f * W_SC).astype(f8)),
        "wo": np.ascontiguousarray((np.asarray(inputs["Wo"], f32) * W_SC).astype(f8)),
        "w1": np.ascontiguousarray(w1_sc.astype(f8 if fc18 else bf16)),
        "w2": np.ascontiguousarray(np.asarray(inputs["W2"], f32).astype(bf16)),
        "bq": ((b1_ln @ wq_f + np.asarray(inputs["bq"], f32).reshape(-1)) * W_SC).reshape(NCD, 128).astype(f32),
        "bk": ((b1_ln @ wk_f + np.asarray(inputs["bk"], f32).reshape(-1)) * W_SC).reshape(NCD, 128).astype(f32),
        "bv": ((b1_ln @ wv_f + np.asarray(inputs["bv"], f32).reshape(-1)) * W_SC).reshape(1, D).astype(bf16),
        "bo": (np.asarray(inputs["bo"], f32) * (C_SC * W_SC)).reshape(1, D).astype(bf16),
        "b1": (b2_ln @ W1 + np.asarray(inputs["b1"], f32)).reshape(NCF, 128).astype(f32),
        "b2": np.asarray(inputs["b2"], f32).reshape(NCD, 128).copy(),
    }
    return out


def kernel(**inputs) -> np.ndarray:
    from concourse.bass_utils import run_bass_kernel_spmd

    no_bias = all(
        not np.any(np.asarray(inputs[k], np.float32))
        for k in ("bv", "bo", "ln1_b"))
    nc = _get_nc("gelu", BPC, MLP_MODE, no_bias)
    w = _prep_weights(inputs, MLP_MODE)
    x = np.asarray(inputs["x"], np.float32)
    # shard over batch, transpose to [b, D, S] per core
    xT = np.ascontiguousarray(
        x.reshape(NCORES, BPC, S, D).swapaxes(2, 3))  # [8, BPC, D, S]
    in_maps = [dict(w, xT=xT[i]) for i in range(NCORES)]
    res = run_bass_kernel_spmd(nc, in_maps, core_ids=list(range(NCORES)))
    outs = [res.results[i]["outT"] for i in range(NCORES)]   # each [BPC, D, S]
    out = np.stack(outs, 0).swapaxes(2, 3).reshape(B, S, D)
    return np.ascontiguousarray(out.astype(np.float32))



# revision 6
# speedup vs baseline: 1.1178x; 1.1178x over previous
"""Trainium2 Bass kernel for a dense transformer block.

Reference math (B=32, S=577, D=768, H=12, DH=64, F=3072, fp32):
  h  = LN1(x);  q,k,v = per-head projections of h
  scores = q @ k^T / sqrt(DH)
  probs  = softmax(scores, axis=QUERY)       # quirk: softmax over the query axis
  attn   = probs @ v;  x2 = x + concat(attn) @ Wo + bo
  out    = x2 + (gelu(LN2(x2) @ W1 + b1) @ W2 + b2)

Strategy: pure data-parallel over batch, 4 batch items per core on 8 cores, no
collectives.  All on-chip activations live in a transposed layout [feature on
partitions, token on free dim].  The attention path (QKV projections, probs@V,
output projection) runs in fp8e4 with DoubleRow perf mode: weights are scaled
x64 and cast to fp8 on the host, activations are quantized on the fly, and
each DoubleRow matmul consumes a 256-deep contraction.  Scores stay bf16.
FC1 fp8 DoubleRow, FC2 bf16.  Residual stream is bf16 on-chip (loaded fp32,
cast once); LN stats matmuls read the bf16 residual directly.

Scheduling is built to keep the PE dense (HAM warm): LN chains are split into
an early part (frees the stats psum) and a late part; next-batch stats run
inside the attention stretch; znorms for both LN streams run under the FC2
tail so FC1/QKV never wait.
"""

import numpy as np
import ml_dtypes

B, S, D, H, DH, F = 32, 577, 768, 12, 64, 3072
NCORES = 8
BPC = B // NCORES          # batches per core
EPS = 1e-5
NCD = D // 128             # 6  d-chunks
NCF = F // 128             # 24 f-chunks
NHP = H // 2               # 6  head pairs
SSPL = [(0, 512), (512, S - 512)]              # free-dim splits of S for matmul/psum
DSPL = [(0, 512), (512, D - 512)]              # free-dim splits of D
TCH = [(i * 128, min(128, S - i * 128)) for i in range((S + 127) // 128)]  # 5 t-chunks
SP = 640                   # fp8 tile row pitch: DoubleRow ldweights needs stride % 64 == 0

W_SC = 64.0      # fp8 weight scale
QK_SC = 64.0     # q/k live at 64x in bf16; folded into the exp input scale
V_SC = 1024.0    # v/rowsum product scale for fp8 storage
C_SC = 32.0      # concat (attn output) fp8 scale

MLP_MODE = "fc1"   # "bf16" | "fc1"

_NC_CACHE = {}


def _build_nc(gelu_kind: str = "gelu", bpc: int = BPC, mlp_mode: str = MLP_MODE, no_bias: bool = False):
    from contextlib import ExitStack
    import concourse.bass as bass
    import concourse.tile as tile
    from concourse import bacc, mybir

    f32, bf16 = mybir.dt.float32, mybir.dt.bfloat16
    f8 = mybir.dt.float8e4
    AF = mybir.ActivationFunctionType
    ALU = mybir.AluOpType
    DR = mybir.MatmulPerfMode.DoubleRow
    GELU = {"gelu": AF.Gelu, "tanh": AF.Tanh}[gelu_kind]
    fc18 = (mlp_mode == "fc1")
    w1dt = f8 if fc18 else bf16
    z2dt = f8 if fc18 else bf16

    nc = bacc.Bacc("TRN2", target_bir_lowering=False, dynamic_dma_scratch_size=2048)
    xT_d = nc.declare_dram_parameter("xT", [bpc, D, S], f32, isOutput=False)
    wq_d = nc.declare_dram_parameter("wq", [D, D], f8, isOutput=False)
    wk_d = nc.declare_dram_parameter("wk", [D, D], f8, isOutput=False)
    wv_d = nc.declare_dram_parameter("wv", [D, D], f8, isOutput=False)
    wo_d = nc.declare_dram_parameter("wo", [D, D], f8, isOutput=False)
    w1_d = nc.declare_dram_parameter("w1", [D, F], w1dt, isOutput=False)
    w2_d = nc.declare_dram_parameter("w2", [F, D], bf16, isOutput=False)
    bq_d = nc.declare_dram_parameter("bq", [NCD, 128], f32, isOutput=False)
    bk_d = nc.declare_dram_parameter("bk", [NCD, 128], f32, isOutput=False)
    bv_d = nc.declare_dram_parameter("bv", [1, D], bf16, isOutput=False)
    bo_d = nc.declare_dram_parameter("bo", [1, D], bf16, isOutput=False)
    b1_d = nc.declare_dram_parameter("b1", [NCF, 128], f32, isOutput=False)
    b2_d = nc.declare_dram_parameter("b2", [NCD, 128], f32, isOutput=False)
    outT_d = nc.declare_dram_parameter("outT", [bpc, D, S], f32, isOutput=True)

    with tile.TileContext(nc) as tc:
        with ExitStack() as ctx:
            wp = ctx.enter_context(tc.tile_pool(name="wp", bufs=1))
            stg = ctx.enter_context(tc.tile_pool(name="stg", bufs=2))     # x fp32 staging
            xbp = ctx.enter_context(tc.tile_pool(name="xbp", bufs=2))     # x residual bf16
            zp = ctx.enter_context(tc.tile_pool(name="zp", bufs=1))       # normalized
            qkp = ctx.enter_context(tc.tile_pool(name="qkp", bufs=1))     # qt/kt/v/concat
            ep = ctx.enter_context(tc.tile_pool(name="ep", bufs=4))       # exp tiles
            gp = ctx.enter_context(tc.tile_pool(name="gp", bufs=1))       # gelu acts
            sp_ = ctx.enter_context(tc.tile_pool(name="sp", bufs=1))      # small stat rows
            tp = ctx.enter_context(tc.tile_pool(name="tp", bufs=1))       # temps
            op_ = ctx.enter_context(tc.tile_pool(name="op", bufs=2))      # out staging
            mmp = ctx.enter_context(tc.tile_pool(name="mmp", bufs=3, space="PSUM"))

            # ---- weights / constants (resident); DMAs deferred until after
            # the first x-shard load so compute starts immediately ----
            wq_s = wp.tile([128, NCD, D], f8, name="wq_s")
            wk_s = wp.tile([128, NCD, D], f8, name="wk_s")
            wv_s = wp.tile([128, NCD, D], f8, name="wv_s")
            wo_s = wp.tile([128, NCD, D], f8, name="wo_s")
            w1_s = wp.tile([128, NCD, F], w1dt, name="w1_s")
            w2_s = wp.tile([128, NCF, D], bf16, name="w2_s")

            def emit_load_weights():
                nc.sync.dma_start(out=wq_s[:, :, :], in_=wq_d.ap().rearrange("(c p) n -> p c n", p=128))
                nc.sync.dma_start(out=wk_s[:, :, :], in_=wk_d.ap().rearrange("(c p) n -> p c n", p=128))
                nc.sync.dma_start(out=wv_s[:, :, :], in_=wv_d.ap().rearrange("(c p) n -> p c n", p=128))
                nc.sync.dma_start(out=wo_s[:, :, :], in_=wo_d.ap().rearrange("(c p) n -> p c n", p=128))
                nc.sync.dma_start(out=w1_s[:, :, :], in_=w1_d.ap().rearrange("(c p) n -> p c n", p=128))
                nc.sync.dma_start(out=w2_s[:, :, :], in_=w2_d.ap().rearrange("(c p) n -> p c n", p=128))
            bqs = wp.tile([128, NCD], f32, name="bqs")
            nc.sync.dma_start(out=bqs[:, :], in_=bq_d.ap().rearrange("c p -> p c"))
            bks = wp.tile([128, NCD], f32, name="bks")
            nc.sync.dma_start(out=bks[:, :], in_=bk_d.ap().rearrange("c p -> p c"))
            bvs = wp.tile([1, D], bf16, name="bvs")
            nc.sync.dma_start(out=bvs[:, :], in_=bv_d[:, :])
            bos = wp.tile([1, D], bf16, name="bos")
            nc.sync.dma_start(out=bos[:, :], in_=bo_d[:, :])
            b1s = wp.tile([128, NCF], f32, name="b1s")
            nc.sync.dma_start(out=b1s[:, :], in_=b1_d.ap().rearrange("c p -> p c"))
            b2s = wp.tile([128, NCD], f32, name="b2s")
            nc.sync.dma_start(out=b2s[:, :], in_=b2_d.ap().rearrange("c p -> p c"))
            ones128 = wp.tile([128, 1], bf16, name="ones128")
            nc.vector.memset(ones128[:, :], 1.0)
            ones1 = wp.tile([1, 128], bf16, name="ones1")
            nc.vector.memset(ones1[:, :], 1.0)
            onesS = wp.tile([1, 512], bf16, name="onesS")
            nc.vector.memset(onesS[:, :], 1.0)
            eps_s = wp.tile([1, 1], f32, name="eps_s")
            nc.vector.memset(eps_s[:, :], EPS)

            # ---------------- per-batch state ----------------
            xbf_tiles = [None] * bpc      # bf16 residual-in [128, NCD, S]
            x2_tiles = [None] * bpc       # bf16 residual-mid
            z1_tiles = [None] * bpc
            qt_tiles = [None] * bpc
            kt_tiles = [None] * bpc
            v_tiles = [None] * bpc
            v8_tiles = [None] * bpc
            z2_tiles = [None] * bpc
            ch1 = [None] * bpc            # (mu_neg, msq_var, w_rstd) rows LN1
            ch2 = [None] * bpc
            chb1 = [None] * bpc           # (rstd_bf, nmr_bf)
            chb2 = [None] * bpc

            def emit_load_x(b):
                xbf = xbp.tile([128, NCD, S], bf16, name="xbf", tag="xbf")
                for c in range(NCD):
                    st = stg.tile([128, S], f32, name="st", tag="stg", bufs=2)
                    nc.sync.dma_start(out=st[:, :],
                                      in_=xT_d[b, c * 128:(c + 1) * 128, :])
                    nc.vector.tensor_scalar_mul(xbf[:, c, :], st[:, :], 1.0)
                xbf_tiles[b] = xbf

            def emit_squares(src_bf, c):
                """bf16 square of one chunk (on the otherwise idle GpSimd)."""
                sq = tp.tile([128, S], bf16, name="sq", tag="sq", bufs=2)
                nc.gpsimd.tensor_mul(sq[:, :], src_bf[:, c, :], src_bf[:, c, :])
                return sq

            def emit_chain_a(spt):
                """Early LN chain: consume the stats psum immediately.
                Rows: A=mu_neg, B=mu2 (later sqrt), C=msq->var (later rstd)."""
                ra = sp_.tile([1, S], f32, name="ra", tag="chf", bufs=6)
                rb = sp_.tile([1, S], f32, name="rb", tag="chf", bufs=6)
                rc = sp_.tile([1, S], f32, name="rc", tag="chf", bufs=6)
                nc.scalar.activation(rc[:, :], spt[32:33, :], AF.Copy, scale=1.0 / D)
                nc.vector.tensor_scalar_mul(ra[:, :], spt[0:1, :], -1.0 / D)
                nc.vector.tensor_mul(rb[:, :], ra[:, :], ra[:, :])   # mu^2
                nc.vector.tensor_sub(rc[:, :], rc[:, :], rb[:, :])   # var
                return (ra, rb, rc)

            def emit_chain_b(ch):
                """Late LN chain: sqrt (ACT table) + reciprocal + bf16 rows."""
                ra, rb, rc = ch
                nc.scalar.activation(rb[:, :], rc[:, :], AF.Sqrt, bias=eps_s[0:1, 0:1])
                nc.vector.reciprocal_approx_fast(rc[:, :], rb[:, :])   # rstd fp32
                rstd_bf = sp_.tile([1, S], bf16, name="rstd_bf", tag="chb", bufs=4)
                nc.vector.tensor_scalar_mul(rstd_bf[:, :], rc[:, :], 1.0)
                nmr_bf = sp_.tile([1, S], bf16, name="nmr_bf", tag="chb", bufs=4)
                nc.vector.tensor_mul(nmr_bf[:, :], ra[:, :], rc[:, :])
                return (rstd_bf, nmr_bf)

            def emit_stats1(b):
                """LN1 stats on the bf16 residual-in + early chain."""
                src = xbf_tiles[b]
                spt = mmp.tile([128, S], f32, name="spt", tag="st", bufs=1,
                               padded_shape=[128, 1024])
                sqs = [emit_squares(src, c) for c in range(NCD)]
                for c in range(NCD):
                    for (s0, sn) in SSPL:
                        nc.tensor.matmul(spt[0:1, s0:s0 + sn], ones128[:, :],
                                         src[:, c, s0:s0 + sn],
                                         start=(c == 0), stop=(c == NCD - 1))
                        nc.tensor.matmul(spt[32:33, s0:s0 + sn], ones128[:, :],
                                         sqs[c][:, s0:s0 + sn],
                                         start=(c == 0), stop=(c == NCD - 1))
                ch1[b] = emit_chain_a(spt)

            def emit_bcast_pair(chb):
                """Broadcast the two [1,S] bf16 rows across 128 partitions via
                rank-1 matmul, then copy psum -> SBUF bf16 for fast DVE reads."""
                rstd_bf, nmr_bf = chb
                out = []
                for row in (rstd_bf, nmr_bf):
                    bc = mmp.tile([128, S], f32, name="bc", tag="mm",
                                  padded_shape=[128, 1024])
                    for (s0, sn) in SSPL:
                        nc.tensor.matmul(bc[:, s0:s0 + sn], ones1[0:1, :],
                                         row[0:1, s0:s0 + sn], start=True, stop=True)
                    sb = tp.tile([128, S], bf16, name="bcb", tag="bcb", bufs=4)
                    nc.vector.tensor_scalar_mul(sb[:, :], bc[:, 0:S], 1.0)
                    out.append(sb)
                return out

            def emit_znorm(src_bf, rbc, nbc, z):
                """z[:, :, 0:S] = src*rstd + nmr, fused over all chunks."""
                ztf = tp.tile([128, NCD, S], bf16, name="ztf", tag="ztf", bufs=1)
                rb3 = rbc[:, 0:S].rearrange("p (o s) -> p o s", o=1).broadcast_to([128, NCD, S])
                nb3 = nbc[:, 0:S].rearrange("p (o s) -> p o s", o=1).broadcast_to([128, NCD, S])
                nc.vector.tensor_mul(ztf[:, :, :], src_bf[:, :, :], rb3)
                nc.vector.tensor_add(z[:, :, 0:S], ztf[:, :, :], nb3)

            def emit_qkv(b):
                z1 = z1_tiles[b]
                qt = qkp.tile([128, NHP, S], bf16, name="qt", tag="qt")
                kt = qkp.tile([128, NHP, S], bf16, name="kt", tag="kt")
                for hp in range(NHP):
                    hc = slice(hp * 128, (hp + 1) * 128)
                    qps = mmp.tile([128, S], f32, name="qps", tag="mm",
                                   padded_shape=[128, 1024])
                    for (s0, sn) in SSPL:
                        for j in range(NCD // 2):
                            nc.tensor.matmul(qps[:, s0:s0 + sn],
                                             wq_s[:, 2 * j:2 * j + 2, hc],
                                             z1[:, 2 * j:2 * j + 2, s0:s0 + sn],
                                             start=(j == 0), stop=(j == NCD // 2 - 1),
                                             perf_mode=DR)
                    nc.vector.tensor_scalar_add(qt[:, hp, :], qps[:, 0:S], bqs[:, hp:hp + 1])
                    kps = mmp.tile([128, S], f32, name="kps", tag="mm",
                                   padded_shape=[128, 1024])
                    for (s0, sn) in SSPL:
                        for j in range(NCD // 2):
                            nc.tensor.matmul(kps[:, s0:s0 + sn],
                                             wk_s[:, 2 * j:2 * j + 2, hc],
                                             z1[:, 2 * j:2 * j + 2, s0:s0 + sn],
                                             start=(j == 0), stop=(j == NCD // 2 - 1),
                                             perf_mode=DR)
                    nc.vector.tensor_scalar_add(kt[:, hp, :], kps[:, 0:S], bks[:, hp:hp + 1])
                # V in natural layout [t, v]; psum holds 64*v, kept in bf16
                v = qkp.tile([128, len(TCH), D], bf16, name="v", tag="v")
                v8 = qkp.tile([128, len(TCH), D], f8, name="v8", tag="v8")
                for it, (t0, tw) in enumerate(TCH):
                    twe = tw + (tw % 2)          # even-M ldweights (pad col is zero)
                    vps = mmp.tile([128, D], f32, name="vps", tag="mm",
                                   padded_shape=[128, 1024])
                    for (s0, sn) in DSPL:
                        for j in range(NCD // 2):
                            nc.tensor.matmul(vps[0:twe, s0:s0 + sn],
                                             z1[:, 2 * j:2 * j + 2, t0:t0 + twe],
                                             wv_s[:, 2 * j:2 * j + 2, s0:s0 + sn],
                                             start=(j == 0),
                                             stop=(no_bias and j == NCD // 2 - 1),
                                             perf_mode=DR)
                        if not no_bias:
                            nc.tensor.matmul(vps[0:twe, s0:s0 + sn], ones1[0:1, 0:twe],
                                             bvs[0:1, s0:s0 + sn], start=False, stop=True)
                    nc.vector.tensor_scalar_mul(v[0:tw, it, :], vps[0:tw, 0:D], 1.0)
                qt_tiles[b], kt_tiles[b] = qt, kt
                v_tiles[b], v8_tiles[b] = v, v8

            def emit_attn_heads(b, interleave=()):
                """interleave: callables emitted between head-pairs to feed the
                PE while ACT grinds the softmax exps."""
                interleave = list(interleave)
                qt, kt = qt_tiles[b], kt_tiles[b]
                v, v8 = v_tiles[b], v8_tiles[b]
                concat = qkp.tile([128, NCD, SP], f8, name="concat", tag="concat")
                for hp in range(NHP):
                    etiles = [None, None]
                    rstiles = [None, None]
                    for h2 in range(2):
                        hb = h2 * 64
                        e = ep.tile([128, len(TCH), SP], f8, name="e", tag="e", bufs=3)
                        rs = sp_.tile([128, len(TCH)], f32, name="rs", tag="rs", bufs=4)
                        for it, (t0, tw) in enumerate(TCH):
                            stps = mmp.tile([128, S], f32, name="stps", tag="mm",
                                            padded_shape=[128, 1024])
                            for (s0, sn) in SSPL:
                                nc.tensor.matmul(stps[0:tw, s0:s0 + sn],
                                                 kt[hb:hb + 64, hp, t0:t0 + tw],
                                                 qt[hb:hb + 64, hp, s0:s0 + sn],
                                                 start=True, stop=True)
                            nc.scalar.activation(e[0:tw, it, 0:S], stps[0:tw, 0:S], AF.Exp,
                                                 bias=0.0,
                                                 scale=float(1.0 / (np.sqrt(DH) * QK_SC * QK_SC)),
                                                 accum_out=rs[0:tw, it:it + 1])
                        etiles[h2] = e
                        rstiles[h2] = rs
                    ap_ps = mmp.tile([128, S], f32, name="ap_ps", tag="mm",
                                     padded_shape=[128, 1024])
                    for h2 in range(2):
                        hb = h2 * 64
                        e, rs = etiles[h2], rstiles[h2]
                        rec = sp_.tile([128, len(TCH)], f32, name="rec", tag="rec", bufs=2)
                        nfull = len(TCH) - 1
                        # v8 = (64*v) * rec * (V_SC/64)  ->  fp8 of V_SC*v/rs
                        nc.vector.reciprocal(rec[:, 0:nfull], rs[:, 0:nfull])
                        nc.vector.tensor_scalar_mul(rec[:, 0:nfull], rec[:, 0:nfull],
                                                    float(V_SC / W_SC))
                        lt0, ltw = TCH[-1]
                        nc.vector.reciprocal(rec[0:ltw, nfull:nfull + 1],
                                             rs[0:ltw, nfull:nfull + 1])
                        nc.vector.tensor_scalar_mul(rec[0:ltw, nfull:nfull + 1],
                                                    rec[0:ltw, nfull:nfull + 1],
                                                    float(V_SC / W_SC))
                        seg = slice(hp * 128 + hb, hp * 128 + hb + 64)
                        rec3 = rec[:, 0:nfull].rearrange("p (c o) -> p c o", o=1) \
                            .broadcast_to([128, nfull, 64])
                        nc.vector.tensor_mul(v8[:, 0:nfull, seg], v[:, 0:nfull, seg], rec3)
                        nc.vector.tensor_scalar_mul(v8[0:ltw, nfull, seg],
                                                    v[0:ltw, nfull, seg],
                                                    rec[0:ltw, nfull:nfull + 1])
                        # DoubleRow dst must start at partition 0, so only the
                        # even head of each pair can pair t-chunks
                        for (s0, sn) in SSPL:
                            if hb == 0:
                                for jt in range(2):
                                    nc.tensor.matmul(ap_ps[hb:hb + 64, s0:s0 + sn],
                                                     v8[:, 2 * jt:2 * jt + 2, seg],
                                                     e[:, 2 * jt:2 * jt + 2, s0:s0 + sn],
                                                     start=(jt == 0), stop=False, perf_mode=DR)
                            else:
                                for it in range(len(TCH) - 1):
                                    nc.tensor.matmul(ap_ps[hb:hb + 64, s0:s0 + sn],
                                                     v8[:, it, seg],
                                                     e[:, it, s0:s0 + sn],
                                                     start=(it == 0), stop=False)
                            nc.tensor.matmul(ap_ps[hb:hb + 64, s0:s0 + sn],
                                             v8[0:ltw, len(TCH) - 1, seg],
                                             e[0:ltw, len(TCH) - 1, s0:s0 + sn],
                                             start=False, stop=True)
                    nc.vector.tensor_scalar_mul(concat[:, hp, 0:S], ap_ps[:, 0:S],
                                                float(C_SC / V_SC))
                    if interleave:
                        interleave.pop(0)()

                # --- output projection + residual; LN2 stats matmuls fused ---
                x2 = xbp.tile([128, NCD, S], bf16, name="x2", tag="x2")
                spt = mmp.tile([128, S], f32, name="spt2", tag="st", bufs=1,
                               padded_shape=[128, 1024])
                xbf = xbf_tiles[b]
                for ec in range(NCD):
                    wops = mmp.tile([128, S], f32, name="wops", tag="mm",
                                    padded_shape=[128, 1024])
                    for (s0, sn) in SSPL:
                        for j in range(NCD // 2):
                            nc.tensor.matmul(wops[:, s0:s0 + sn],
                                             wo_s[:, 2 * j:2 * j + 2, ec * 128:(ec + 1) * 128],
                                             concat[:, 2 * j:2 * j + 2, s0:s0 + sn],
                                             start=(j == 0),
                                             stop=(no_bias and j == NCD // 2 - 1),
                                             perf_mode=DR)
                        if not no_bias:
                            # + bo * (C_SC*W_SC) via rank-1
                            nc.tensor.matmul(wops[:, s0:s0 + sn],
                                             bos[0:1, ec * 128:(ec + 1) * 128],
                                             onesS[0:1, 0:sn], start=False, stop=True)
                    nc.vector.scalar_tensor_tensor(x2[:, ec, :], wops[:, 0:S],
                                                   float(1.0 / (C_SC * W_SC)),
                                                   xbf[:, ec, :],
                                                   op0=ALU.mult, op1=ALU.add)
                    sq = emit_squares(x2, ec)
                    for (s0, sn) in SSPL:
                        nc.tensor.matmul(spt[0:1, s0:s0 + sn], ones128[:, :],
                                         x2[:, ec, s0:s0 + sn],
                                         start=(ec == 0), stop=(ec == NCD - 1))
                        nc.tensor.matmul(spt[32:33, s0:s0 + sn], ones128[:, :],
                                         sq[:, s0:s0 + sn],
                                         start=(ec == 0), stop=(ec == NCD - 1))
                x2_tiles[b] = x2
                ch2[b] = emit_chain_a(spt)

            def emit_fc1(b, g, lo, hi):
                z2 = z2_tiles[b]
                gsc = (1.0 / W_SC) if fc18 else 1.0
                for fc in range(lo, hi):
                    fps = mmp.tile([128, S], f32, name="fps", tag="mm",
                                   padded_shape=[128, 1024])
                    if fc18:
                        for (s0, sn) in SSPL:
                            for j in range(NCD // 2):
                                nc.tensor.matmul(fps[:, s0:s0 + sn],
                                                 w1_s[:, 2 * j:2 * j + 2, fc * 128:(fc + 1) * 128],
                                                 z2[:, 2 * j:2 * j + 2, s0:s0 + sn],
                                                 start=(j == 0), stop=(j == NCD // 2 - 1),
                                                 perf_mode=DR)
                    else:
                        for (s0, sn) in SSPL:
                            for c in range(NCD):
                                nc.tensor.matmul(fps[:, s0:s0 + sn],
                                                 w1_s[:, c, fc * 128:(fc + 1) * 128],
                                                 z2[:, c, s0:s0 + sn],
                                                 start=(c == 0), stop=(c == NCD - 1))
                    nc.scalar.activation(g[:, fc, :], fps[:, 0:S], GELU,
                                         bias=b1s[:, fc:fc + 1], scale=gsc)

            def emit_fc2_chunk(b, g, ec):
                p2 = mmp.tile([128, S], f32, name="p2", tag="mm",
                              padded_shape=[128, 1024])
                for (s0, sn) in SSPL:
                    for fc in range(NCF):
                        nc.tensor.matmul(p2[:, s0:s0 + sn],
                                         w2_s[:, fc, ec * 128:(ec + 1) * 128],
                                         g[:, fc, s0:s0 + sn],
                                         start=(fc == 0), stop=(fc == NCF - 1))
                ot = op_.tile([128, S], f32, name="ot", tag="ostg", bufs=2)
                nc.vector.scalar_tensor_tensor(ot[:, :], p2[:, 0:S],
                                               b2s[:, ec:ec + 1],
                                               x2_tiles[b][:, ec, :],
                                               op0=ALU.add, op1=ALU.add)
                nc.sync.dma_start(out=outT_d[b, ec * 128:(ec + 1) * 128, :],
                                  in_=ot[:, :])

            # ---------------- emission schedule ----------------
            emit_load_x(0)
            emit_load_weights()
            emit_load_x(1)
            emit_stats1(0)
            chb1[0] = emit_chain_b(ch1[0])

            def _zfin1_from_chb(b):
                rbc, nbc = emit_bcast_pair(chb1[b])
                z1 = zp.tile([128, NCD, SP], f8, name="z1", tag="z1", bufs=2)
                nc.vector.memset(z1[:, :, S:S + 1], 0.0)
                emit_znorm(xbf_tiles[b], rbc, nbc, z1)
                z1_tiles[b] = z1

            _zfin1_from_chb(0)
            emit_qkv(0)
            il0 = [lambda: emit_stats1(1)]
            emit_attn_heads(0, interleave=il0)
            chb1[1] = emit_chain_b(ch1[1])
            chb2[0] = emit_chain_b(ch2[0])
            _zfin1_from_chb(1)

            def _zfin2_from_chb(b):
                rbc, nbc = emit_bcast_pair(chb2[b])
                z2 = zp.tile([128, NCD, SP if fc18 else S], z2dt, name="z2", tag="z2",
                             bufs=1)
                if fc18:
                    nc.vector.memset(z2[:, :, S:S + 1], 0.0)
                emit_znorm(x2_tiles[b], rbc, nbc, z2)
                z2_tiles[b] = z2

            _zfin2_from_chb(0)
            for b in range(bpc):
                if b + 2 < bpc:
                    emit_load_x(b + 2)
                g = gp.tile([128, NCF, S], bf16, name="g", tag="g")
                emit_fc1(b, g, 0, 8)
                if b + 1 < bpc:
                    emit_qkv(b + 1)
                emit_fc1(b, g, 8, NCF)
                if b + 1 < bpc:
                    il = [(lambda ec=ec: emit_fc2_chunk(b, g, ec)) for ec in range(3)]
                    if b + 2 < bpc:
                        il.append(lambda: emit_stats1(b + 2))
                    il.append(lambda: emit_fc2_chunk(b, g, 3))
                    emit_attn_heads(b + 1, interleave=il)
                    if b + 2 < bpc:
                        chb1[b + 2] = emit_chain_b(ch1[b + 2])
                    chb2[b + 1] = emit_chain_b(ch2[b + 1])
                    emit_fc2_chunk(b, g, 4)
                    _zfin2_from_chb(b + 1)
                    if b + 2 < bpc:
                        _zfin1_from_chb(b + 2)
                    emit_fc2_chunk(b, g, 5)
                else:
                    for ec in range(NCD):
                        emit_fc2_chunk(b, g, ec)
    nc.finalize()
    return nc


def _get_nc(gelu_kind: str = "gelu", bpc: int = BPC, mlp_mode: str = MLP_MODE,
            no_bias: bool = False):
    key = (gelu_kind, bpc, mlp_mode, no_bias)
    if key not in _NC_CACHE:
        _NC_CACHE[key] = _build_nc(gelu_kind, bpc, mlp_mode, no_bias)
    return _NC_CACHE[key]


def _prep_weights(inputs, mlp_mode: str = MLP_MODE):
    bf16 = ml_dtypes.bfloat16
    f8 = ml_dtypes.float8_e4m3
    f32 = np.float32
    fc18 = (mlp_mode == "fc1")
    Wq, Wk, Wv = inputs["Wq"], inputs["Wk"], inputs["Wv"]
    g1, b1_ln = np.asarray(inputs["ln1_g"], f32), np.asarray(inputs["ln1_b"], f32)
    g2, b2_ln = np.asarray(inputs["ln2_g"], f32), np.asarray(inputs["ln2_b"], f32)

    def flat(Wx):  # [H, D, DH] -> [D, H*DH]
        return np.ascontiguousarray(np.transpose(np.asarray(Wx, f32), (1, 0, 2)).reshape(D, D))

    wq_f, wk_f, wv_f = flat(Wq), flat(Wk), flat(Wv)
    W1 = np.asarray(inputs["W1"], f32)
    w1_sc = (g2[:, None] * W1) * (W_SC if fc18 else 1.0)
    out = {
        "wq": np.ascontiguousarray((g1[:, None] * wq_f * W_SC).astype(f8)),
        "wk": np.ascontiguousarray((g1[:, None] * wk_f * W_SC).astype(f8)),
        "wv": np.ascontiguousarray((g1[:, None] * wv_f * W_SC).astype(f8)),
        "wo": np.ascontiguousarray((np.asarray(inputs["Wo"], f32) * W_SC).astype(f8)),
        "w1": np.ascontiguousarray(w1_sc.astype(f8 if fc18 else bf16)),
        "w2": np.ascontiguousarray(np.asarray(inputs["W2"], f32).astype(bf16)),
        "bq": ((b1_ln @ wq_f + np.asarray(inputs["bq"], f32).reshape(-1)) * W_SC).reshape(NCD, 128).astype(f32),
        "bk": ((b1_ln @ wk_f + np.asarray(inputs["bk"], f32).reshape(-1)) * W_SC).reshape(NCD, 128).astype(f32),
        "bv": ((b1_ln @ wv_f + np.asarray(inputs["bv"], f32).reshape(-1)) * W_SC).reshape(1, D).astype(bf16),
        "bo": (np.asarray(inputs["bo"], f32) * (C_SC * W_SC)).reshape(1, D).astype(bf16),
        "b1": (b2_ln @ W1 + np.asarray(inputs["b1"], f32)).reshape(NCF, 128).astype(f32),
        "b2": np.asarray(inputs["b2"], f32).reshape(NCD, 128).copy(),
    }
    return out


def kernel(**inputs) -> np.ndarray:
    from concourse.bass_utils import run_bass_kernel_spmd

    no_bias = all(
        not np.any(np.asarray(inputs[k], np.float32))
        for k in ("bv", "bo", "ln1_b"))
    nc = _get_nc("gelu", BPC, MLP_MODE, no_bias)
    w = _prep_weights(inputs, MLP_MODE)
    x = np.asarray(inputs["x"], np.float32)
    # shard over batch, transpose to [b, D, S] per core
    xT = np.ascontiguousarray(
        x.reshape(NCORES, BPC, S, D).swapaxes(2, 3))  # [8, BPC, D, S]
    in_maps = [dict(w, xT=xT[i]) for i in range(NCORES)]
    res = run_bass_kernel_spmd(nc, in_maps, core_ids=list(range(NCORES)))
    outs = [res.results[i]["outT"] for i in range(NCORES)]   # each [BPC, D, S]
    out = np.stack(outs, 0).swapaxes(2, 3).reshape(B, S, D)
    return np.ascontiguousarray(out.astype(np.float32))


# revision 53
# speedup vs baseline: 1.2498x; 1.1181x over previous
"""Trainium2 Bass kernel for a dense transformer block.

Reference math (B=32, S=577, D=768, H=12, DH=64, F=3072, fp32):
  h  = LN1(x);  q,k,v = per-head projections of h
  scores = q @ k^T / sqrt(DH)
  probs  = softmax(scores, axis=QUERY)       # quirk: softmax over the query axis
  attn   = probs @ v;  x2 = x + concat(attn) @ Wo + bo
  out    = x2 + (gelu(LN2(x2) @ W1 + b1) @ W2 + b2)

Strategy: pure data-parallel over batch, 4 batch items per core on 8 cores, no
collectives.  All on-chip activations live in a transposed layout [feature on
partitions, token on free dim].  The attention path (QKV projections, probs@V,
output projection) runs in fp8e4 with DoubleRow perf mode: weights are scaled
x64 and cast to fp8 on the host, activations are quantized on the fly, and
each DoubleRow matmul consumes a 256-deep contraction.  Scores stay bf16.
FC1 fp8 DoubleRow, FC2 bf16.  Residual stream is bf16 on-chip (loaded fp32,
cast once); LN stats matmuls read the bf16 residual directly.

Scheduling is built to keep the PE dense (HAM warm): LN chains are split into
an early part (frees the stats psum) and a late part; next-batch stats run
inside the attention stretch; znorms for both LN streams run under the FC2
tail so FC1/QKV never wait.
"""

import numpy as np
import ml_dtypes

B, S, D, H, DH, F = 32, 577, 768, 12, 64, 3072
NCORES = 8
BPC = B // NCORES          # batches per core
EPS = 1e-5
NCD = D // 128             # 6  d-chunks
NCF = F // 128             # 24 f-chunks
NHP = H // 2               # 6  head pairs
SSPL = [(0, 512), (512, S - 512)]              # free-dim splits of S for matmul/psum
DSPL = [(0, 512), (512, D - 512)]              # free-dim splits of D
TCH = [(i * 128, min(128, S - i * 128)) for i in range((S + 127) // 128)]  # 5 t-chunks
SP = 640                   # fp8 tile row pitch: DoubleRow ldweights needs stride % 64 == 0

W_SC = 64.0      # fp8 weight scale
QK_SC = 64.0     # q/k live at 64x in bf16; folded into the exp input scale
V_SC = 1024.0    # v/rowsum product scale for fp8 storage
C_SC = 32.0      # concat (attn output) fp8 scale

MLP_MODE = "fc1"   # "bf16" | "fc1"

_NC_CACHE = {}


def _build_nc(gelu_kind: str = "gelu", bpc: int = BPC, mlp_mode: str = MLP_MODE, no_bias: bool = False):
    from contextlib import ExitStack
    import concourse.bass as bass
    import concourse.tile as tile
    from concourse import bacc, mybir

    f32, bf16 = mybir.dt.float32, mybir.dt.bfloat16
    f8 = mybir.dt.float8e4
    AF = mybir.ActivationFunctionType
    ALU = mybir.AluOpType
    DR = mybir.MatmulPerfMode.DoubleRow
    GELU = {"gelu": AF.Gelu, "tanh": AF.Tanh}[gelu_kind]
    fc18 = (mlp_mode == "fc1")
    w1dt = f8 if fc18 else bf16
    z2dt = f8 if fc18 else bf16

    nc = bacc.Bacc("TRN2", target_bir_lowering=False, dynamic_dma_scratch_size=2048)
    xT_d = nc.declare_dram_parameter("xT", [bpc, D, S], f32, isOutput=False)
    wq_d = nc.declare_dram_parameter("wq", [D, D], f8, isOutput=False)
    wk_d = nc.declare_dram_parameter("wk", [D, D], f8, isOutput=False)
    wv_d = nc.declare_dram_parameter("wv", [D, D], f8, isOutput=False)
    wo_d = nc.declare_dram_parameter("wo", [D, D], f8, isOutput=False)
    w1_d = nc.declare_dram_parameter("w1", [D, F], w1dt, isOutput=False)
    w2_d = nc.declare_dram_parameter("w2", [F, D], bf16, isOutput=False)
    bq_d = nc.declare_dram_parameter("bq", [NCD, 128], f32, isOutput=False)
    bk_d = nc.declare_dram_parameter("bk", [NCD, 128], f32, isOutput=False)
    bv_d = nc.declare_dram_parameter("bv", [1, D], bf16, isOutput=False)
    bo_d = nc.declare_dram_parameter("bo", [1, D], bf16, isOutput=False)
    b1_d = nc.declare_dram_parameter("b1", [NCF, 128], f32, isOutput=False)
    b2_d = nc.declare_dram_parameter("b2", [NCD, 128], f32, isOutput=False)
    outT_d = nc.declare_dram_parameter("outT", [bpc, D, S], f32, isOutput=True)

    with tile.TileContext(nc) as tc:
        with ExitStack() as ctx:
            wp = ctx.enter_context(tc.tile_pool(name="wp", bufs=1))
            stg = ctx.enter_context(tc.tile_pool(name="stg", bufs=2))     # x fp32 staging
            xbp = ctx.enter_context(tc.tile_pool(name="xbp", bufs=2))     # x residual bf16
            zp = ctx.enter_context(tc.tile_pool(name="zp", bufs=1))       # normalized
            qkp = ctx.enter_context(tc.tile_pool(name="qkp", bufs=1))     # qt/kt/v/concat
            ep = ctx.enter_context(tc.tile_pool(name="ep", bufs=4))       # exp tiles
            gp = ctx.enter_context(tc.tile_pool(name="gp", bufs=1))       # gelu acts
            sp_ = ctx.enter_context(tc.tile_pool(name="sp", bufs=1))      # small stat rows
            tp = ctx.enter_context(tc.tile_pool(name="tp", bufs=1))       # temps
            op_ = ctx.enter_context(tc.tile_pool(name="op", bufs=2))      # out staging
            mmp = ctx.enter_context(tc.tile_pool(name="mmp", bufs=3, space="PSUM"))

            # ---- weights / constants (resident); DMAs deferred until after
            # the first x-shard load so compute starts immediately ----
            wq_s = wp.tile([128, NCD, D], f8, name="wq_s")
            wk_s = wp.tile([128, NCD, D], f8, name="wk_s")
            wv_s = wp.tile([128, NCD, D], f8, name="wv_s")
            wo_s = wp.tile([128, NCD, D], f8, name="wo_s")
            w1_s = wp.tile([128, NCD, F], w1dt, name="w1_s")
            w2_s = wp.tile([128, NCF, D], bf16, name="w2_s")

            def emit_load_weights():
                nc.sync.dma_start(out=wq_s[:, :, :], in_=wq_d.ap().rearrange("(c p) n -> p c n", p=128))
                nc.sync.dma_start(out=wk_s[:, :, :], in_=wk_d.ap().rearrange("(c p) n -> p c n", p=128))
                nc.sync.dma_start(out=wv_s[:, :, :], in_=wv_d.ap().rearrange("(c p) n -> p c n", p=128))
                nc.sync.dma_start(out=wo_s[:, :, :], in_=wo_d.ap().rearrange("(c p) n -> p c n", p=128))
                nc.sync.dma_start(out=w1_s[:, :, :], in_=w1_d.ap().rearrange("(c p) n -> p c n", p=128))
                nc.sync.dma_start(out=w2_s[:, :, :], in_=w2_d.ap().rearrange("(c p) n -> p c n", p=128))
            bqs = wp.tile([128, NCD], f32, name="bqs")
            nc.sync.dma_start(out=bqs[:, :], in_=bq_d.ap().rearrange("c p -> p c"))
            bks = wp.tile([128, NCD], f32, name="bks")
            nc.sync.dma_start(out=bks[:, :], in_=bk_d.ap().rearrange("c p -> p c"))
            bvs = wp.tile([1, D], bf16, name="bvs")
            nc.sync.dma_start(out=bvs[:, :], in_=bv_d[:, :])
            bos = wp.tile([1, D], bf16, name="bos")
            nc.sync.dma_start(out=bos[:, :], in_=bo_d[:, :])
            b1s = wp.tile([128, NCF], f32, name="b1s")
            nc.sync.dma_start(out=b1s[:, :], in_=b1_d.ap().rearrange("c p -> p c"))
            b2s = wp.tile([128, NCD], f32, name="b2s")
            nc.sync.dma_start(out=b2s[:, :], in_=b2_d.ap().rearrange("c p -> p c"))
            ones128 = wp.tile([128, 1], bf16, name="ones128")
            nc.vector.memset(ones128[:, :], 1.0)
            ones1 = wp.tile([1, 128], bf16, name="ones1")
            nc.vector.memset(ones1[:, :], 1.0)
            onesS = wp.tile([1, 512], bf16, name="onesS")
            nc.vector.memset(onesS[:, :], 1.0)
            eps_s = wp.tile([1, 1], f32, name="eps_s")
            nc.vector.memset(eps_s[:, :], EPS)

            # ---------------- per-batch state ----------------
            xbf_tiles = [None] * bpc      # bf16 residual-in [128, NCD, S]
            x2_tiles = [None] * bpc       # bf16 residual-mid
            z1_tiles = [None] * bpc
            qt_tiles = [None] * bpc
            kt_tiles = [None] * bpc
            v_tiles = [None] * bpc
            v8_tiles = [None] * bpc
            z2_tiles = [None] * bpc
            ch1 = [None] * bpc            # (mu_neg, msq_var, w_rstd) rows LN1
            ch2 = [None] * bpc
            chb1 = [None] * bpc           # (rstd_bf, nmr_bf)
            chb2 = [None] * bpc

            def emit_load_x(b):
                xbf = xbp.tile([128, NCD, S], bf16, name="xbf", tag="xbf")
                for c in range(NCD):
                    st = stg.tile([128, S], f32, name="st", tag="stg", bufs=2)
                    nc.sync.dma_start(out=st[:, :],
                                      in_=xT_d[b, c * 128:(c + 1) * 128, :])
                    nc.vector.tensor_scalar_mul(xbf[:, c, :], st[:, :], 1.0)
                xbf_tiles[b] = xbf

            def emit_squares(src_bf, c):
                """bf16 square of one chunk (on the otherwise idle GpSimd)."""
                sq = tp.tile([128, S], bf16, name="sq", tag="sq", bufs=2)
                nc.gpsimd.tensor_mul(sq[:, :], src_bf[:, c, :], src_bf[:, c, :])
                return sq

            def emit_chain_a(spt):
                """Early LN chain: consume the stats psum immediately.
                Rows: A=mu_neg, B=mu2 (later sqrt), C=msq->var (later rstd)."""
                ra = sp_.tile([1, S], f32, name="ra", tag="chf", bufs=6)
                rb = sp_.tile([1, S], f32, name="rb", tag="chf", bufs=6)
                rc = sp_.tile([1, S], f32, name="rc", tag="chf", bufs=6)
                nc.scalar.activation(rc[:, :], spt[32:33, :], AF.Copy, scale=1.0 / D)
                nc.vector.tensor_scalar_mul(ra[:, :], spt[0:1, :], -1.0 / D)
                nc.vector.tensor_mul(rb[:, :], ra[:, :], ra[:, :])   # mu^2
                nc.vector.tensor_sub(rc[:, :], rc[:, :], rb[:, :])   # var
                return (ra, rb, rc)

            def emit_chain_b(ch):
                """Late LN chain: sqrt (ACT table) + reciprocal + bf16 rows."""
                ra, rb, rc = ch
                nc.scalar.activation(rb[:, :], rc[:, :], AF.Sqrt, bias=eps_s[0:1, 0:1])
                nc.vector.reciprocal_approx_fast(rc[:, :], rb[:, :])   # rstd fp32
                rstd_bf = sp_.tile([1, S], bf16, name="rstd_bf", tag="chb", bufs=4)
                nc.vector.tensor_scalar_mul(rstd_bf[:, :], rc[:, :], 1.0)
                nmr_bf = sp_.tile([1, S], bf16, name="nmr_bf", tag="chb", bufs=4)
                nc.vector.tensor_mul(nmr_bf[:, :], ra[:, :], rc[:, :])
                return (rstd_bf, nmr_bf)

            def emit_stats1(b):
                """LN1 stats on the bf16 residual-in + early chain."""
                src = xbf_tiles[b]
                spt = mmp.tile([128, S], f32, name="spt", tag="p2", bufs=1,
                               padded_shape=[128, 1024])
                sqs = [emit_squares(src, c) for c in range(NCD)]
                for c in range(NCD):
                    for (s0, sn) in SSPL:
                        nc.tensor.matmul(spt[0:1, s0:s0 + sn], ones128[:, :],
                                         src[:, c, s0:s0 + sn],
                                         start=(c == 0), stop=(c == NCD - 1))
                        nc.tensor.matmul(spt[32:33, s0:s0 + sn], ones128[:, :],
                                         sqs[c][:, s0:s0 + sn],
                                         start=(c == 0), stop=(c == NCD - 1))
                ch1[b] = emit_chain_a(spt)

            def emit_bcast_pair(chb):
                """Broadcast the two [1,S] bf16 rows across 128 partitions on
                the (otherwise idle) GpSimd engine - no PE or DVE involvement,
                so the boundary znorms never stall the matmul stream."""
                rstd_bf, nmr_bf = chb
                out = []
                for row in (rstd_bf, nmr_bf):
                    bc = mmp.tile([128, S], f32, name="bc", tag="mm",
                                  padded_shape=[128, 1024])
                    for (s0, sn) in SSPL:
                        nc.tensor.matmul(bc[:, s0:s0 + sn], ones1[0:1, :],
                                         row[0:1, s0:s0 + sn], start=True, stop=True)
                    sb = tp.tile([128, S], bf16, name="bcb", tag="bcb", bufs=4)
                    nc.vector.tensor_scalar_mul(sb[:, :], bc[:, 0:S], 1.0)
                    out.append(sb)
                return out

            def emit_znorm(src_bf, rbc, nbc, z):
                """z[:, :, 0:S] = src*rstd + nmr, fused over half the chunks at
                a time so downstream matmuls can start on the first chunks."""
                for (lo, hi) in ((0, NCD // 2), (NCD // 2, NCD)):
                    w = hi - lo
                    ztf = tp.tile([128, NCD // 2, S], bf16, name="ztf", tag="ztf", bufs=2)
                    rb3 = rbc[:, 0:S].rearrange("p (o s) -> p o s", o=1).broadcast_to([128, w, S])
                    nb3 = nbc[:, 0:S].rearrange("p (o s) -> p o s", o=1).broadcast_to([128, w, S])
                    nc.vector.tensor_mul(ztf[:, 0:w, :], src_bf[:, lo:hi, :], rb3)
                    nc.vector.tensor_add(z[:, lo:hi, 0:S], ztf[:, 0:w, :], nb3)

            def _qkv_steps(b, v_early=False):
                z1 = z1_tiles[b]
                qt = qkp.tile([128, NHP, S], bf16, name="qt", tag="qt")
                kt = qkp.tile([128, NHP, S], bf16, name="kt", tag="kt")
                v = qkp.tile([128, len(TCH), D], bf16, name="v", tag="v")
                v8 = qkp.tile([128, len(TCH), D], f8, name="v8", tag="v8")
                qt_tiles[b], kt_tiles[b] = qt, kt
                v_tiles[b], v8_tiles[b] = v, v8

                def qk(hp):
                    hc = slice(hp * 128, (hp + 1) * 128)
                    qps = mmp.tile([128, S], f32, name="qps", tag="mm",
                                   padded_shape=[128, 1024])
                    for (s0, sn) in SSPL:
                        for j in range(NCD // 2):
                            nc.tensor.matmul(qps[:, s0:s0 + sn],
                                             wq_s[:, 2 * j:2 * j + 2, hc],
                                             z1[:, 2 * j:2 * j + 2, s0:s0 + sn],
                                             start=(j == 0), stop=(j == NCD // 2 - 1),
                                             perf_mode=DR)
                    nc.vector.tensor_scalar_add(qt[:, hp, :], qps[:, 0:S], bqs[:, hp:hp + 1])
                    kps = mmp.tile([128, S], f32, name="kps", tag="mm",
                                   padded_shape=[128, 1024])
                    for (s0, sn) in SSPL:
                        for j in range(NCD // 2):
                            nc.tensor.matmul(kps[:, s0:s0 + sn],
                                             wk_s[:, 2 * j:2 * j + 2, hc],
                                             z1[:, 2 * j:2 * j + 2, s0:s0 + sn],
                                             start=(j == 0), stop=(j == NCD // 2 - 1),
                                             perf_mode=DR)
                    nc.vector.tensor_scalar_add(kt[:, hp, :], kps[:, 0:S], bks[:, hp:hp + 1])

                def vchunk(it):
                    t0, tw = TCH[it]
                    twe = tw + (tw % 2)          # even-M ldweights (pad col is zero)
                    vps = mmp.tile([128, D], f32, name="vps", tag="mm",
                                   padded_shape=[128, 1024])
                    for (s0, sn) in DSPL:
                        for j in range(NCD // 2):
                            nc.tensor.matmul(vps[0:twe, s0:s0 + sn],
                                             z1[:, 2 * j:2 * j + 2, t0:t0 + twe],
                                             wv_s[:, 2 * j:2 * j + 2, s0:s0 + sn],
                                             start=(j == 0),
                                             stop=(no_bias and j == NCD // 2 - 1),
                                             perf_mode=DR)
                        if not no_bias:
                            nc.tensor.matmul(vps[0:twe, s0:s0 + sn], ones1[0:1, 0:twe],
                                             bvs[0:1, s0:s0 + sn], start=False, stop=True)
                    nc.vector.tensor_scalar_mul(v[0:tw, it, :], vps[0:tw, 0:D], 1.0)

                if v_early:
                    qk(0)
                    yield
                    for it in range(len(TCH)):
                        vchunk(it)
                        yield
                    for hp in range(1, NHP):
                        qk(hp)
                        yield
                else:
                    for hp in range(NHP):
                        qk(hp)
                        yield
                    for it in range(len(TCH)):
                        vchunk(it)
                        yield

            def emit_qkv(b):
                for _ in _qkv_steps(b):
                    pass

            def _pad_fill(pad_ps, n):
                """Dummy matmuls into the unused pad columns of the live psum
                tile: keeps the PE activity monitor warm (full clock) through
                exp-paced stretches that have no real fill work."""
                if pad_ps is None:
                    return
                for _ in range(min(n, 2)):
                    nc.tensor.matmul(pad_ps[0:1, 650:1024], ones1[0:1, 0:1],
                                     onesS[0:1, 0:374], start=True, stop=True)

            class GenFiller:
                """Adapter: streams a generator of emission steps through the
                attention fill points (used for the first batch, where the
                next batch's LN1 stats serve as the PE filler).  Once the
                generator is exhausted, emits dummy matmuls into a dedicated
                dead psum tile to keep the PE clock warm."""

                def __init__(self, gen):
                    self.gen = gen
                    self.done = False
                    self.wu = None

                def quantum(self, n, pad_ps=None):
                    if self.done:
                        return
                    try:
                        next(self.gen)
                    except StopIteration:
                        self.done = True

                def drain(self):
                    while not self.done:
                        self.quantum(1)

            class FC2Filler:
                """Streams one batch's FC2 matmuls in small quanta between the
                attention matmuls so the PE never idles on exp latency (keeps
                HAM at full clock).  Uses a dedicated PSUM slot (tag p2) so the
                in-flight FC2 accumulator never blocks the attn psum ring."""

                def __init__(self, b, g, ecs):
                    self.b, self.g = b, g
                    self.work = [(ec, s0, sn, fc)
                                 for ec in ecs for (s0, sn) in SSPL for fc in range(NCF)]
                    self.pos = 0
                    self.p2 = None
                    self.cur_ec = None

                def _epilogue(self, ec):
                    ot = op_.tile([128, S], f32, name="ot", tag="ostg", bufs=2)
                    nc.vector.scalar_tensor_tensor(ot[:, :], self.p2[:, 0:S],
                                                   b2s[:, ec:ec + 1],
                                                   x2_tiles[self.b][:, ec, :],
                                                   op0=ALU.add, op1=ALU.add)
                    nc.sync.dma_start(out=outT_d[self.b, ec * 128:(ec + 1) * 128, :],
                                      in_=ot[:, :])
                    self.p2 = None
                    self.cur_ec = None

                def quantum(self, n, pad_ps=None):
                    if self.pos >= len(self.work):
                        return
                    end = min(self.pos + n, len(self.work))
                    while self.pos < end:
                        ec, s0, sn, fc = self.work[self.pos]
                        if self.cur_ec != ec:
                            self.p2 = mmp.tile([128, 1024], f32, name="p2f",
                                               tag="p2", bufs=1)
                            self.cur_ec = ec
                        nc.tensor.matmul(self.p2[:, s0:s0 + sn],
                                         w2_s[:, fc, ec * 128:(ec + 1) * 128],
                                         self.g[:, fc, s0:s0 + sn],
                                         start=(fc == 0), stop=(fc == NCF - 1))
                        self.pos += 1
                        done_ec = (self.pos == len(self.work)
                                   or self.work[self.pos][0] != ec)
                        if done_ec:
                            self._epilogue(ec)

                def drain(self):
                    self.quantum(len(self.work) - self.pos)

            def emit_attn_heads(b, filler=None, post_heads=None):
                """filler: FC2Filler streamed between attention matmul groups.
                post_heads: emitted after the head loop, before the output
                projection (next-batch LN1 stats land here, PE-dense)."""
                def fill(n, pad_ps=None):
                    if filler is not None:
                        filler.quantum(n, pad_ps)
                qt, kt = qt_tiles[b], kt_tiles[b]
                v, v8 = v_tiles[b], v8_tiles[b]
                concat = qkp.tile([128, NCD, SP], f8, name="concat", tag="concat")
                for hp in range(NHP):
                    etiles = [None, None]
                    rstiles = [None, None]
                    for h2 in range(2):
                        hb = h2 * 64
                        e = ep.tile([128, len(TCH), SP], f8, name="e", tag="e", bufs=4)
                        rs = sp_.tile([128, len(TCH)], f32, name="rs", tag="rs", bufs=8)
                        for it, (t0, tw) in enumerate(TCH):
                            stps = mmp.tile([128, 1024], f32, name="stps", tag="mm")
                            for (s0, sn) in SSPL:
                                nc.tensor.matmul(stps[0:tw, s0:s0 + sn],
                                                 kt[hb:hb + 64, hp, t0:t0 + tw],
                                                 qt[hb:hb + 64, hp, s0:s0 + sn],
                                                 start=True, stop=True)
                            nc.scalar.activation(e[0:tw, it, 0:S], stps[0:tw, 0:S], AF.Exp,
                                                 bias=0.0,
                                                 scale=float(1.0 / (np.sqrt(DH) * QK_SC * QK_SC)),
                                                 accum_out=rs[0:tw, it:it + 1])
                            if it >= 2:
                                fill(2, stps)
                        fill(2, stps)
                        etiles[h2] = e
                        rstiles[h2] = rs
                    ap_ps = mmp.tile([128, 1024], f32, name="ap_ps", tag="mm")
                    for h2 in range(2):
                        hb = h2 * 64
                        e, rs = etiles[h2], rstiles[h2]
                        rec = sp_.tile([128, len(TCH)], f32, name="rec", tag="rec", bufs=4)
                        nfull = len(TCH) - 1
                        # v8 = (64*v) * rec * (V_SC/64)  ->  fp8 of V_SC*v/rs
                        nc.vector.reciprocal(rec[:, 0:nfull], rs[:, 0:nfull])
                        nc.vector.tensor_scalar_mul(rec[:, 0:nfull], rec[:, 0:nfull],
                                                    float(V_SC / W_SC))
                        lt0, ltw = TCH[-1]
                        nc.vector.reciprocal(rec[0:ltw, nfull:nfull + 1],
                                             rs[0:ltw, nfull:nfull + 1])
                        nc.vector.tensor_scalar_mul(rec[0:ltw, nfull:nfull + 1],
                                                    rec[0:ltw, nfull:nfull + 1],
                                                    float(V_SC / W_SC))
                        seg = slice(hp * 128 + hb, hp * 128 + hb + 64)
                        rec3 = rec[:, 0:nfull].rearrange("p (c o) -> p c o", o=1) \
                            .broadcast_to([128, nfull, 64])
                        nc.vector.tensor_mul(v8[:, 0:nfull, seg], v[:, 0:nfull, seg], rec3)
                        nc.vector.tensor_scalar_mul(v8[0:ltw, nfull, seg],
                                                    v[0:ltw, nfull, seg],
                                                    rec[0:ltw, nfull:nfull + 1])
                        # DoubleRow dst must start at partition 0, so only the
                        # even head of each pair can pair t-chunks
                        for (s0, sn) in SSPL:
                            if hb == 0:
                                for jt in range(2):
                                    nc.tensor.matmul(ap_ps[hb:hb + 64, s0:s0 + sn],
                                                     v8[:, 2 * jt:2 * jt + 2, seg],
                                                     e[:, 2 * jt:2 * jt + 2, s0:s0 + sn],
                                                     start=(jt == 0), stop=False, perf_mode=DR)
                            else:
                                for it in range(len(TCH) - 1):
                                    nc.tensor.matmul(ap_ps[hb:hb + 64, s0:s0 + sn],
                                                     v8[:, it, seg],
                                                     e[:, it, s0:s0 + sn],
                                                     start=(it == 0), stop=False)
                            nc.tensor.matmul(ap_ps[hb:hb + 64, s0:s0 + sn],
                                             v8[0:ltw, len(TCH) - 1, seg],
                                             e[0:ltw, len(TCH) - 1, s0:s0 + sn],
                                             start=False, stop=True)
                        fill(8, ap_ps)
                    nc.vector.tensor_scalar_mul(concat[:, hp, 0:S], ap_ps[:, 0:S],
                                                float(C_SC / V_SC))

                if filler is not None:
                    filler.drain()
                if post_heads is not None:
                    post_heads()

                # --- output projection + residual; LN2 stats matmuls fused ---
                x2 = xbp.tile([128, NCD, S], bf16, name="x2", tag="x2")
                spt = mmp.tile([128, S], f32, name="spt2", tag="p2", bufs=1,
                               padded_shape=[128, 1024])
                xbf = xbf_tiles[b]
                for ec in range(NCD):
                    wops = mmp.tile([128, S], f32, name="wops", tag="mm",
                                    padded_shape=[128, 1024])
                    for (s0, sn) in SSPL:
                        for j in range(NCD // 2):
                            nc.tensor.matmul(wops[:, s0:s0 + sn],
                                             wo_s[:, 2 * j:2 * j + 2, ec * 128:(ec + 1) * 128],
                                             concat[:, 2 * j:2 * j + 2, s0:s0 + sn],
                                             start=(j == 0),
                                             stop=(no_bias and j == NCD // 2 - 1),
                                             perf_mode=DR)
                        if not no_bias:
                            # + bo * (C_SC*W_SC) via rank-1
                            nc.tensor.matmul(wops[:, s0:s0 + sn],
                                             bos[0:1, ec * 128:(ec + 1) * 128],
                                             onesS[0:1, 0:sn], start=False, stop=True)
                    nc.vector.scalar_tensor_tensor(x2[:, ec, :], wops[:, 0:S],
                                                   float(1.0 / (C_SC * W_SC)),
                                                   xbf[:, ec, :],
                                                   op0=ALU.mult, op1=ALU.add)
                    sq = emit_squares(x2, ec)
                    for (s0, sn) in SSPL:
                        nc.tensor.matmul(spt[0:1, s0:s0 + sn], ones128[:, :],
                                         x2[:, ec, s0:s0 + sn],
                                         start=(ec == 0), stop=(ec == NCD - 1))
                        nc.tensor.matmul(spt[32:33, s0:s0 + sn], ones128[:, :],
                                         sq[:, s0:s0 + sn],
                                         start=(ec == 0), stop=(ec == NCD - 1))
                x2_tiles[b] = x2
                ch2[b] = emit_chain_a(spt)

            def emit_fc1(b, g, lo, hi):
                z2 = z2_tiles[b]
                gsc = (1.0 / W_SC) if fc18 else 1.0
                for fc in range(lo, hi):
                    # last two psums use the spare slot so the attention psum
                    # ring that follows doesn't wait on the tail gelus
                    fps = mmp.tile([128, S], f32, name="fps",
                                   tag=("p2" if fc >= NCF - 2 else "mm"), bufs=(1 if fc >= NCF - 2 else None),
                                   padded_shape=[128, 1024])
                    if fc18:
                        for (s0, sn) in SSPL:
                            for j in range(NCD // 2):
                                nc.tensor.matmul(fps[:, s0:s0 + sn],
                                                 w1_s[:, 2 * j:2 * j + 2, fc * 128:(fc + 1) * 128],
                                                 z2[:, 2 * j:2 * j + 2, s0:s0 + sn],
                                                 start=(j == 0), stop=(j == NCD // 2 - 1),
                                                 perf_mode=DR)
                    else:
                        for (s0, sn) in SSPL:
                            for c in range(NCD):
                                nc.tensor.matmul(fps[:, s0:s0 + sn],
                                                 w1_s[:, c, fc * 128:(fc + 1) * 128],
                                                 z2[:, c, s0:s0 + sn],
                                                 start=(c == 0), stop=(c == NCD - 1))
                    nc.scalar.activation(g[:, fc, :], fps[:, 0:S], GELU,
                                         bias=b1s[:, fc:fc + 1], scale=gsc)

            def emit_fc2_chunk(b, g, ec):
                p2 = mmp.tile([128, S], f32, name="p2", tag="mm",
                              padded_shape=[128, 1024])
                for (s0, sn) in SSPL:
                    for fc in range(NCF):
                        nc.tensor.matmul(p2[:, s0:s0 + sn],
                                         w2_s[:, fc, ec * 128:(ec + 1) * 128],
                                         g[:, fc, s0:s0 + sn],
                                         start=(fc == 0), stop=(fc == NCF - 1))
                ot = op_.tile([128, S], f32, name="ot", tag="ostg", bufs=2)
                nc.vector.scalar_tensor_tensor(ot[:, :], p2[:, 0:S],
                                               b2s[:, ec:ec + 1],
                                               x2_tiles[b][:, ec, :],
                                               op0=ALU.add, op1=ALU.add)
                nc.sync.dma_start(out=outT_d[b, ec * 128:(ec + 1) * 128, :],
                                  in_=ot[:, :])

            # ---------------- emission schedule ----------------
            # dummy matmuls to pre-warm the PE clock (HAM) during the initial
            # x DMA, so the first real matmuls run at full clock
            wu = mmp.tile([128, 512], f32, name="wu", tag="mm",
                          padded_shape=[128, 1024])
            for _ in range(24):
                nc.tensor.matmul(wu[0:1, 0:512], ones1[0:1, 0:1],
                                 onesS[0:1, 0:512], start=True, stop=True)
            emit_load_x(0)
            emit_load_weights()
            emit_load_x(1)
            emit_stats1(0)
            chb1[0] = emit_chain_b(ch1[0])

            def _zfin1_from_chb(b):
                rbc, nbc = emit_bcast_pair(chb1[b])
                z1 = zp.tile([128, NCD, SP], f8, name="z1", tag="z1", bufs=1)
                nc.vector.memset(z1[:, :, S:S + 1], 0.0)
                emit_znorm(xbf_tiles[b], rbc, nbc, z1)
                z1_tiles[b] = z1

            _zfin1_from_chb(0)
            qkv0_steps = _qkv_steps(0, v_early=True)
            for _ in range(6):
                next(qkv0_steps)          # q/k head pair 0 + all of V

            def _prologue_fill():
                yield from qkv0_steps     # rest of qkv(0) streams into attn(0)
                emit_stats1(1)
                yield
                chb1[1] = emit_chain_b(ch1[1])
                yield
                _zfin1_from_chb(1)
                yield

            emit_attn_heads(0, filler=GenFiller(_prologue_fill()))
            chb2[0] = emit_chain_b(ch2[0])

            def _zfin2_from_chb(b):
                rbc, nbc = emit_bcast_pair(chb2[b])
                z2 = zp.tile([128, NCD, SP if fc18 else S], z2dt, name="z2", tag="z2",
                             bufs=1)
                if fc18:
                    nc.vector.memset(z2[:, :, S:S + 1], 0.0)
                emit_znorm(x2_tiles[b], rbc, nbc, z2)
                z2_tiles[b] = z2

            _zfin2_from_chb(0)
            for b in range(bpc):
                if b + 2 < bpc:
                    emit_load_x(b + 2)
                g = gp.tile([128, NCF, S], bf16, name="g", tag="g")
                emit_fc1(b, g, 0, 8)
                if b + 1 < bpc:
                    emit_qkv(b + 1)
                emit_fc1(b, g, 8, NCF)
                if b + 1 < bpc:
                    filler = FC2Filler(b, g, range(4))
                    ph = (lambda: emit_stats1(b + 2)) if b + 2 < bpc else None
                    emit_attn_heads(b + 1, filler=filler, post_heads=ph)
                    if b + 2 < bpc:
                        chb1[b + 2] = emit_chain_b(ch1[b + 2])
                    chb2[b + 1] = emit_chain_b(ch2[b + 1])
                    emit_fc2_chunk(b, g, 4)
                    _zfin2_from_chb(b + 1)
                    if b + 2 < bpc:
                        _zfin1_from_chb(b + 2)
                    emit_fc2_chunk(b, g, 5)
                else:
                    for ec in range(NCD):
                        emit_fc2_chunk(b, g, ec)
    nc.finalize()
    return nc


def _get_nc(gelu_kind: str = "gelu", bpc: int = BPC, mlp_mode: str = MLP_MODE,
            no_bias: bool = False):
    key = (gelu_kind, bpc, mlp_mode, no_bias)
    if key not in _NC_CACHE:
        _NC_CACHE[key] = _build_nc(gelu_kind, bpc, mlp_mode, no_bias)
    return _NC_CACHE[key]


def _prep_weights(inputs, mlp_mode: str = MLP_MODE):
    bf16 = ml_dtypes.bfloat16
    f8 = ml_dtypes.float8_e4m3
    f32 = np.float32
    fc18 = (mlp_mode == "fc1")
    Wq, Wk, Wv = inputs["Wq"], inputs["Wk"], inputs["Wv"]
    g1, b1_ln = np.asarray(inputs["ln1_g"], f32), np.asarray(inputs["ln1_b"], f32)
    g2, b2_ln = np.asarray(inputs["ln2_g"], f32), np.asarray(inputs["ln2_b"], f32)

    def flat(Wx):  # [H, D, DH] -> [D, H*DH]
        return np.ascontiguousarray(np.transpose(np.asarray(Wx, f32), (1, 0, 2)).reshape(D, D))

    wq_f, wk_f, wv_f = flat(Wq), flat(Wk), flat(Wv)
    W1 = np.asarray(inputs["W1"], f32)
    w1_sc = (g2[:, None] * W1) * (W_SC if fc18 else 1.0)
    out = {
        "wq": np.ascontiguousarray((g1[:, None] * wq_f * W_SC).astype(f8)),
        "wk": np.ascontiguousarray((g1[:, None] * wk_f * W_SC).astype(f8)),
        "wv": np.ascontiguousarray((g1[:, None] * wv_f * W_SC).astype(f8)),
        "wo": np.ascontiguousarray((np.asarray(inputs["Wo"], f32) * W_SC).astype(f8)),
        "w1": np.ascontiguousarray(w1_sc.astype(f8 if fc18 else bf16)),
        "w2": np.ascontiguousarray(np.asarray(inputs["W2"], f32).astype(bf16)),
        "bq": ((b1_ln @ wq_f + np.asarray(inputs["bq"], f32).reshape(-1)) * W_SC).reshape(NCD, 128).astype(f32),
        "bk": ((b1_ln @ wk_f + np.asarray(inputs["bk"], f32).reshape(-1)) * W_SC).reshape(NCD, 128).astype(f32),
        "bv": ((b1_ln @ wv_f + np.asarray(inputs["bv"], f32).reshape(-1)) * W_SC).reshape(1, D).astype(bf16),
        "bo": (np.asarray(inputs["bo"], f32) * (C_SC * W_SC)).reshape(1, D).astype(bf16),
        "b1": (b2_ln @ W1 + np.asarray(inputs["b1"], f32)).reshape(NCF, 128).astype(f32),
        "b2": np.asarray(inputs["b2"], f32).reshape(NCD, 128).copy(),
    }
    return out


def kernel(**inputs) -> np.ndarray:
    from concourse.bass_utils import run_bass_kernel_spmd

    no_bias = all(
        not np.any(np.asarray(inputs[k], np.float32))
        for k in ("bv", "bo", "ln1_b"))
    nc = _get_nc("gelu", BPC, MLP_MODE, no_bias)
    w = _prep_weights(inputs, MLP_MODE)
    x = np.asarray(inputs["x"], np.float32)
    # shard over batch, transpose to [b, D, S] per core
    xT = np.ascontiguousarray(
        x.reshape(NCORES, BPC, S, D).swapaxes(2, 3))  # [8, BPC, D, S]
    in_maps = [dict(w, xT=xT[i]) for i in range(NCORES)]
    res = run_bass_kernel_spmd(nc, in_maps, core_ids=list(range(NCORES)))
    outs = [res.results[i]["outT"] for i in range(NCORES)]   # each [BPC, D, S]
    out = np.stack(outs, 0).swapaxes(2, 3).reshape(B, S, D)
    return np.ascontiguousarray(out.astype(np.float32))
